# revision 20
# baseline (speedup 1.0000x reference)
"""Self-contained Trainium2 kernel for nn_CausalSelfAttention_65309272703394.

Sharding: tensor-parallel over heads across 8 cores. Core c computes q-heads
{2c, 2c+1} and kv-head c//2, runs causal GQA attention + xsa orthogonalization
for those heads, and produces a partial output projection (full [S, D] shape).
Host sums the 8 partials; v output taken from cores 0/2/4/6.

Matmuls run as float32r (fp32 layout, 11-bit mantissa, 1 cycle/row for
N>=256); elementwise math is fp32.
"""

import numpy as np

# Model / sharding constants (hardcoded per contract)
B, S, D = 1, 2048, 2048
H, KV = 16, 4
HD = D // H          # 128
G = H // KV          # 4
NCORES = 8
NH = 2               # q heads per core
P = 128
NS = S // P          # 16 s-tiles
ND = D // P          # 16 d-tiles
SQC = 512            # sq chunk width
NCH = S // SQC       # 4 chunks
ROPE_BASE = 10000.0
BIG = 1.0e30
EPS = float(np.finfo(np.float32).eps)

_CACHE = {}


def _build_program():
    import concourse.mybir as mybir
    import concourse.tile as tile
    from concourse import bacc

    f32 = mybir.dt.float32
    f32r = mybir.dt.float32r
    AF = mybir.ActivationFunctionType
    OP = mybir.AluOpType

    nc = bacc.Bacc()

    # ---- I/O (f32r inputs are pre-rounded fp32 bits from the host) ----
    xtb = nc.dram_tensor("xtb", [NS, P, ND, P], f32r, kind="ExternalInput")
    wt = nc.dram_tensor("wt", [P, ND, 512], f32r, kind="ExternalInput")
    wpt = nc.dram_tensor("wpt", [P, NH, D], f32r, kind="ExternalInput")
    trig = nc.dram_tensor("trig", [P, NS, 2, HD // 2], f32, kind="ExternalInput")
    gains = nc.dram_tensor("gains", [P, NH], f32, kind="ExternalInput")
    ident = nc.dram_tensor("ident", [P, P], f32r, kind="ExternalInput")
    lle = nc.dram_tensor("lle", [P, P], f32r, kind="ExternalInput")
    bwide = nc.dram_tensor("bwide", [P, 1024], f32r, kind="ExternalInput")
    out_part = nc.dram_tensor("out_part", [S, D], f32, kind="ExternalOutput")
    v_out = nc.dram_tensor("v_out", [S, HD], f32, kind="ExternalOutput")
    sscratch = nc.dram_tensor(
        "sscratch", [NH * NCH, SQC], f32, kind="ExternalOutput"
    )

    with tile.TileContext(nc) as tc:
        with (
            tc.tile_pool(name="singles", bufs=1) as singles,
            tc.tile_pool(name="bigs", bufs=1) as bigs,
        ):
            # ---- constants ----
            wpt_sb = singles.tile([P, NH, D], f32r)
            nc.sync.dma_start(wpt_sb, wpt[:])
            trig_sb = singles.tile([P, NS, 2, HD // 2], f32)
            nc.sync.dma_start(trig_sb, trig[:])
            gains_sb = singles.tile([P, NH], f32)
            nc.sync.dma_start(gains_sb, gains[:])
            ident_sb = singles.tile([P, P], f32r)
            nc.sync.dma_start(ident_sb, ident[:])
            lle_sb = singles.tile([P, P], f32r)
            nc.sync.dma_start(lle_sb, lle[:])
            bwide_sb = singles.tile([P, 1024], f32r)
            nc.sync.dma_start(bwide_sb, bwide[:])
            # ones vector for column-sum matmuls: last column of lle is all 1
            ones_col = lle_sb[:, P - 1:P]
            eps_sb = singles.tile([P, 1], f32)
            nc.vector.memset(eps_sb, EPS)
            zero_sb = singles.tile([P, 1], f32)
            nc.vector.memset(zero_sb, 0.0)

            # ---- whole-kernel tensors ----
            qt = bigs.tile([P, NH, S], f32r)      # rope'd+scaled q, transposed
            kt = bigs.tile([P, S], f32r)          # rope'd k, transposed
            v_nat = bigs.tile([P, NS, HD], f32r)  # v natural
            ssq = bigs.tile([P, NS, 4], f32)      # sum-of-squares q0,q1,k,v
            rstat = bigs.tile([P, NS, 3], f32)    # 1/rms q0,q1 (w/ gain), k
            rv2 = bigs.tile([P, NS], f32)         # 1/|v|^2
            y2t = bigs.tile([P, NH, S], f32r)     # final y''T

            # ================= Phase 1: QKV projections =================
            # processed in two si-halves of 8 to bound SBUF staging
            HS = NS // 2
            with (
                tc.tile_pool(name="ph1", bufs=3) as ph1,
                tc.tile_pool(name="ph1b", bufs=1) as ph1b,
                tc.tile_pool(name="ph1_psum", bufs=3, space="PSUM") as ph1_ps,
                tc.tile_pool(name="tr_psum", bufs=2, space="PSUM") as tr_ps,
            ):
                wt_sb = ph1b.tile([P, ND, 512], f32r)
                nc.sync.dma_start(wt_sb, wt[:])

                for half in range(2):
                    s0 = half * HS
                    qk_sb = ph1b.tile([P, HS, 384], f32, tag="qk")
                    rot = ph1b.tile([P, HS, 3, HD], f32r, tag="rot")

                    for sl_ in range(HS):
                        si = s0 + sl_
                        xt = ph1.tile([P, ND, P], f32r, tag="xt")
                        nc.sync.dma_start(xt, xtb[si])
                        qkv_ps = ph1_ps.tile([P, 512], f32, tag="qkv")
                        for dt in range(ND):
                            nc.tensor.matmul(
                                qkv_ps, xt[:, dt, :], wt_sb[:, dt, :],
                                start=(dt == 0), stop=(dt == ND - 1),
                            )
                        # v natural + v output
                        nc.scalar.copy(v_nat[:, si, :], qkv_ps[:, 384:512])
                        nc.sync.dma_start(
                            v_out[si * P:(si + 1) * P, :],
                            v_nat[:, si, :].bitcast(f32),
                        )
                        # stage raw q|k to SBUF for batched rope
                        nc.scalar.copy(qk_sb[:, sl_, :], qkv_ps[:, 0:384])
                        # sum of squares per slab (ACT square with accum)
                        for sl in range(4):
                            junk = ph1.tile([P, P], f32, tag="sqj")
                            nc.scalar.activation(
                                junk, qkv_ps[:, sl * P:(sl + 1) * P], AF.Square,
                                bias=zero_sb[:, 0:1],
                                accum_out=ssq[:, si, sl:sl + 1],
                            )

                    # ---- stats: rstat = 1/sqrt(ms+eps), rv2 = 1/sum(v^2) ----
                    std3 = ph1b.tile([P, HS, 3], f32, tag="std3")
                    nc.scalar.activation(
                        std3, ssq[:, s0:s0 + HS, 0:3], AF.Sqrt,
                        scale=1.0 / HD, bias=eps_sb[:, 0:1],
                    )
                    nc.vector.reciprocal(rstat[:, s0:s0 + HS, :], std3)
                    nc.vector.reciprocal(
                        rv2[:, s0:s0 + HS], ssq[:, s0:s0 + HS, 3]
                    )
                    # fold (gain * hd^-0.5) into q columns of rstat
                    nc.vector.tensor_tensor(
                        rstat[:, s0:s0 + HS, 0:2], rstat[:, s0:s0 + HS, 0:2],
                        gains_sb[:, None, :].to_broadcast((P, HS, NH)),
                        OP.mult,
                    )

                    # ---- batched rope over this half ----
                    qv = qk_sb[:, :, 0:256].rearrange(
                        "p s (h two f) -> p s h two f", two=2, f=64
                    )
                    kv_ = qk_sb[:, :, 256:384].rearrange(
                        "p s (h two f) -> p s h two f", two=2, f=64
                    )
                    rq = rot[:, :, 0:2, :].rearrange(
                        "p s h (two f) -> p s h two f", two=2, f=64
                    )
                    rk = rot[:, :, 2:3, :].rearrange(
                        "p s h (two f) -> p s h two f", two=2, f=64
                    )
                    trig_h = trig_sb[:, s0:s0 + HS]
                    for (src, dst, nh) in ((qv, rq, NH), (kv_, rk, 1)):
                        a = src[:, :, :, 0, :]
                        b = src[:, :, :, 1, :]
                        cos = trig_h[:, :, None, 0, :].to_broadcast(
                            (P, HS, nh, 64))
                        sin = trig_h[:, :, None, 1, :].to_broadcast(
                            (P, HS, nh, 64))
                        ta = ph1b.tile([P, HS, nh, 64], f32, tag=f"ta{nh}")
                        tb = ph1b.tile([P, HS, nh, 64], f32, tag=f"tb{nh}")
                        nc.vector.tensor_tensor(ta, a, cos, OP.mult)
                        nc.vector.tensor_tensor(tb, b, sin, OP.mult)
                        nc.vector.tensor_tensor(
                            dst[:, :, :, 0, :], ta, tb, OP.add)
                        ta2 = ph1b.tile([P, HS, nh, 64], f32, tag=f"ta{nh}")
                        tb2 = ph1b.tile([P, HS, nh, 64], f32, tag=f"tb{nh}")
                        nc.vector.tensor_tensor(ta2, a, sin, OP.mult)
                        nc.vector.tensor_tensor(tb2, b, cos, OP.mult)
                        nc.vector.tensor_tensor(
                            dst[:, :, :, 1, :], tb2, ta2, OP.subtract
                        )
                    # scale q by rstat (in place)
                    nc.vector.tensor_tensor(
                        rot[:, :, 0:2, :], rot[:, :, 0:2, :],
                        rstat[:, s0:s0 + HS, 0:2, None].to_broadcast(
                            (P, HS, 2, HD)),
                        OP.mult,
                    )

                    # ---- transposes: rot/v_nat -> qt/kt ----
                    jobs = []
                    for h in range(NH):
                        jobs += [(rot[:, sl_, h, :], qt[:, h, :], s0 + sl_)
                                 for sl_ in range(HS)]
                    jobs += [(rot[:, sl_, 2, :], kt[:, :], s0 + sl_)
                             for sl_ in range(HS)]
                    for g0 in range(0, len(jobs), 4):
                        tp = tr_ps.tile([P, 512], f32r, tag="tr")
                        for j, (src, dst, si) in enumerate(jobs[g0:g0 + 4]):
                            nc.tensor.matmul(
                                tp[:, j * P:(j + 1) * P], src,
                                ident_sb, is_transpose=True,
                            )
                        dst, si0 = jobs[g0][1], jobs[g0][2]
                        nc.scalar.copy(dst[:, si0 * P:si0 * P + 512], tp)

            # ================= Phase 2: attention =================
            with (
                tc.tile_pool(name="ph2", bufs=6) as ph2,
                tc.tile_pool(name="ph2b", bufs=1) as ph2b,
                tc.tile_pool(name="ph2c", bufs=2) as ph2c,
                tc.tile_pool(name="s_psum", bufs=2, space="PSUM") as s_ps_pool,
                tc.tile_pool(name="y_psum", bufs=2, space="PSUM") as y_ps_pool,
                tc.tile_pool(name="r_psum", bufs=1, space="PSUM") as r_ps_pool,
                tc.tile_pool(name="t2_psum", bufs=2, space="PSUM") as t2_ps_pool,
            ):
                y_nat = ph2b.tile([P, NS, NH, HD], f32)
                sums_nat = ph2b.tile([P, NH, NCH, 4], f32)
                for h in range(NH):
                    for c in range(NCH):
                        nt = 4 * c + 4
                        y_ps = y_ps_pool.tile([P, SQC], f32, tag="y")
                        sum_ps = r_ps_pool.tile([1, SQC], f32, tag="sum")
                        for t in range(nt):
                            diag = t >= 4 * c
                            s_ps = s_ps_pool.tile([P, SQC], f32, tag="s")
                            nc.tensor.matmul(
                                s_ps,
                                kt[:, t * P:(t + 1) * P],
                                qt[:, h, c * SQC:(c + 1) * SQC],
                                start=True, stop=not diag,
                            )
                            if diag:
                                rr = t - 4 * c
                                nc.tensor.matmul(
                                    s_ps, lle_sb,
                                    bwide_sb[:, 512 - 128 * rr:1024 - 128 * rr],
                                    start=False, stop=True,
                                )
                            et = ph2.tile([P, SQC], f32r, tag="et")
                            nc.scalar.activation(
                                et, s_ps, AF.Exp, bias=zero_sb[:, 0:1],
                                scale=rstat[:, t, 2:3],
                            )
                            nc.tensor.matmul(
                                y_ps, v_nat[:, t, :], et,
                                start=(t == 0), stop=(t == nt - 1),
                            )
                            nc.tensor.matmul(
                                sum_ps, ones_col, et,
                                start=(t == 0), stop=(t == nt - 1),
                            )
                        # stage yraw chunk and transpose to natural
                        yrow = ph2c.tile([P, SQC], f32r, tag="yrow")
                        nc.scalar.copy(yrow, y_ps)
                        tp = t2_ps_pool.tile([P, SQC], f32r, tag="ytr")
                        for j in range(4):
                            nc.tensor.matmul(
                                tp[:, j * P:(j + 1) * P],
                                yrow[:, j * P:(j + 1) * P], ident_sb,
                                is_transpose=True,
                            )
                        nc.scalar.copy(y_nat[:, 4 * c:4 * c + 4, h, :], tp)
                        # sums row -> natural layout via DRAM round-trip
                        srow = ph2c.tile([1, SQC], f32, tag="srow")
                        nc.scalar.copy(srow, sum_ps)
                        hc = h * NCH + c
                        nc.sync.dma_start(sscratch[hc:hc + 1, :], srow[0:1, :])
                        nc.sync.dma_start(
                            sums_nat[:, h, c, :],
                            sscratch[hc].rearrange("(si sp) -> sp si", sp=P),
                        )

                # ---- ortho + normalize (natural layout) ----
                rs_nat = ph2b.tile([P, NH, NCH, 4], f32)
                nc.vector.reciprocal(rs_nat, sums_nat)
                dots = ph2b.tile([P, NS, NH], f32)
                for h in range(NH):
                    for si in range(NS):
                        junk = ph2.tile([P, HD], f32, tag="dj")
                        nc.vector.scalar_tensor_tensor(
                            junk, y_nat[:, si, h, :], 1.0, v_nat[:, si, :],
                            OP.mult, OP.mult,
                            accum_out=dots[:, si, h:h + 1],
                        )
                # d3 = -dots * rv2
                d3 = ph2b.tile([P, NS, NH], f32)
                nc.vector.tensor_tensor(
                    d3, dots, rv2[:, :, None].to_broadcast((P, NS, NH)), OP.mult
                )
                nc.vector.tensor_scalar_mul(d3, d3, -1.0)
                # y'' = (yraw + v * (-d3)) * rs  per (si, h)
                y2n = ph2b.tile([P, NS, NH, HD], f32r)
                rs_v = rs_nat.rearrange("p h c f -> p h (c f)")
                for h in range(NH):
                    for si in range(NS):
                        nc.vector.scalar_tensor_tensor(
                            y2n[:, si, h, :], v_nat[:, si, :],
                            d3[:, si, h:h + 1], y_nat[:, si, h, :],
                            OP.mult, OP.add,
                        )
                        nc.vector.tensor_scalar_mul(
                            y2n[:, si, h, :], y2n[:, si, h, :],
                            rs_v[:, h, si:si + 1],
                        )
                # transpose y'' -> y2t
                for h in range(NH):
                    for g0 in range(0, NS, 4):
                        tp = t2_ps_pool.tile([P, SQC], f32r, tag="ytr")
                        for j in range(4):
                            nc.tensor.matmul(
                                tp[:, j * P:(j + 1) * P],
                                y2n[:, g0 + j, h, :], ident_sb,
                                is_transpose=True,
                            )
                        nc.scalar.copy(y2t[:, h, g0 * P:g0 * P + 512], tp)

            # ================= Phase 3: output projection =================
            with (
                tc.tile_pool(name="ph3", bufs=4) as ph3,
                tc.tile_pool(name="o_psum", bufs=4, space="PSUM") as o_ps_pool,
            ):
                for si in range(NS):
                    for nchunk in range(NCH):
                        o_ps = o_ps_pool.tile([P, SQC], f32, tag="o")
                        for mt in range(NH):
                            nc.tensor.matmul(
                                o_ps,
                                y2t[:, mt, si * P:(si + 1) * P],
                                wpt_sb[:, mt, nchunk * SQC:(nchunk + 1) * SQC],
                                start=(mt == 0), stop=(mt == NH - 1),
                            )
                        ot = ph3.tile([P, SQC], f32, tag="ot")
                        if (si + nchunk) % 2 == 0:
                            nc.scalar.copy(ot, o_ps)
                        else:
                            nc.vector.tensor_copy(ot, o_ps)
                        nc.sync.dma_start(
                            out_part[si * P:(si + 1) * P,
                                     nchunk * SQC:(nchunk + 1) * SQC],
                            ot,
                        )

    _split_matmul_waits(nc, mybir)
    nc.compile()
    return nc


def _split_matmul_waits(nc, mybir):
    """f32r self-loading Matmult takes at most one sync wait (and has no
    ldweights for bacc to move waits onto); hoist extra waits onto adjacent
    same-engine NoOps inserted right before the matmul."""
    n = 0
    for fn in nc.m.functions:
        for blk in fn.blocks:
            out = []
            for inst in blk.instructions:
                si = getattr(inst, "sync_info", None)
                if (
                    isinstance(inst, mybir.InstMatmult)
                    and si is not None
                    and si.on_wait is not None
                    and len(si.on_wait) > 1
                ):
                    waits = list(si.on_wait)
                    for j, w in enumerate(waits[:-1]):
                        nop = mybir.InstNoOp(
                            name=f"{inst.name}-w{j}",
                            engine=inst.engine,
                            ins=[],
                            outs=[],
                            sync_info=mybir.SyncInfo(
                                on_wait=[w], on_update=[]
                            ),
                        )
                        nc.register_instruction(nop)
                        out.append(nop)
                    inst.sync_info = mybir.SyncInfo(
                        on_wait=[waits[-1]], on_update=list(si.on_update or [])
                    )
                    n += 1
                out.append(inst)
            blk.instructions = out
    return n


def _round_f32r(a):
    """Round fp32 array to fp32r (11-bit mantissa, RNE) like the HW expects."""
    u = np.ascontiguousarray(a, dtype=np.float32).view(np.uint32)
    u = (u + 0x800 + ((u >> 12) & 1)) & np.uint32(0xFFFFF000)
    return u.view(np.float32)


def _host_inputs(x, Wq, Wk, Wv, Wp, q_gain):
    """Build the 8 per-core input dicts."""
    x2 = np.ascontiguousarray(x.reshape(S, D), dtype=np.float32)
    # xtb[si, dp, dt, sp] = x[si*128+sp, dt*128+dp]
    xtb = _round_f32r(x2.reshape(NS, P, ND, P).transpose(0, 3, 2, 1))

    # trig tables
    inv_freq = 1.0 / (ROPE_BASE ** (np.arange(0, HD, 2, dtype=np.float32) / HD))
    pos = np.arange(S, dtype=np.float32)[:, None] * inv_freq[None, :]
    cos = np.cos(pos).astype(np.float32).reshape(NS, P, HD // 2)
    sin = np.sin(pos).astype(np.float32).reshape(NS, P, HD // 2)
    trig = np.empty((P, NS, 2, HD // 2), np.float32)
    trig[:, :, 0, :] = cos.transpose(1, 0, 2)
    trig[:, :, 1, :] = sin.transpose(1, 0, 2)
    trig = np.ascontiguousarray(trig)

    ident = np.eye(P, dtype=np.float32)
    k_idx = np.arange(P)[:, None]
    i_idx = np.arange(P)[None, :]
    lle = (k_idx <= i_idx).astype(np.float32)
    bwide = np.zeros((P, 1024), np.float32)
    bwide[:, 0:512] = -BIG
    m_idx = np.arange(512, 1024)
    kk = m_idx - 511
    valid = (kk >= 1) & (kk < P)
    bwide[kk[valid], m_idx[valid]] = -BIG
    bwide = _round_f32r(bwide)

    in_maps = []
    for c in range(NCORES):
        h0 = NH * c
        g = c // 2
        wq_c = Wq[h0 * HD:(h0 + NH) * HD, :]        # [256, D]
        wk_g = Wk[g * HD:(g + 1) * HD, :]           # [128, D]
        wv_g = Wv[g * HD:(g + 1) * HD, :]           # [128, D]
        wt = np.empty((P, ND, 512), np.float32)
        # wt[dp, dt, m] = W[m, dt*128+dp]
        wt[:, :, 0:256] = wq_c.T.reshape(ND, P, NH * HD).transpose(1, 0, 2)
        wt[:, :, 256:384] = wk_g.T.reshape(ND, P, HD).transpose(1, 0, 2)
        wt[:, :, 384:512] = wv_g.T.reshape(ND, P, HD).transpose(1, 0, 2)

        # wpt[mp, mt, n] = Wp[n, h0*HD + mt*128 + mp]
        wp_c = Wp[:, h0 * HD:(h0 + NH) * HD]        # [D, 256]
        wpt = _round_f32r(wp_c.T.reshape(NH, P, D).transpose(1, 0, 2))

        gains = np.broadcast_to(
            (q_gain[h0:h0 + NH] * (HD ** -0.5)).astype(np.float32)[None, :],
            (P, NH),
        )
        in_maps.append({
            "xtb": xtb,
            "wt": _round_f32r(wt),
            "wpt": wpt,
            "trig": trig,
            "gains": np.ascontiguousarray(gains),
            "ident": ident,
            "lle": lle,
            "bwide": bwide,
        })
    return in_maps


def kernel(x, Wq, Wk, Wv, Wp, q_gain, _trace=False):
    from concourse import bass_utils

    if "nc" not in _CACHE:
        _CACHE["nc"] = _build_program()
    nc = _CACHE["nc"]

    in_maps = _host_inputs(
        np.asarray(x), np.asarray(Wq), np.asarray(Wk), np.asarray(Wv),
        np.asarray(Wp), np.asarray(q_gain),
    )
    res = bass_utils.run_bass_kernel_spmd(
        nc, in_maps, core_ids=list(range(NCORES)), trace=_trace,
    )
    _CACHE["last_result"] = res
    outs = res.results
    out = np.zeros((S, D), np.float64)
    for c in range(NCORES):
        out += outs[c]["out_part"].astype(np.float64)
    out = out.astype(np.float32).reshape(B, S, D)
    v = np.stack([outs[2 * g]["v_out"] for g in range(KV)], axis=0)
    v = v.reshape(B, KV, S, HD).astype(np.float32)
    return (out, v)
